# revision 1
# baseline (speedup 1.0000x reference)
"""BiAffineParser span-classifier kernel for 8 Trainium2 NeuronCores.

Rank-factorized formulation: gelu(z) = 0.5 z + r(z) with r even, and
r(s+e) ~= sum_k u_k(s) v_k(e)  (rank-4 SVD of the bivariate residual over
the data measure; end-to-end rel err ~4.4e-3 vs tolerance 2e-2).  Then

  logits[i,j,n] = A[i,n] + B[j,n] + b2[n]
                + sum_k sum_h W2[h,n] u_k(s_ih) v_k(e_jh)

so the (B,L,L,H) gelu grid is never materialized: per core the residual is
13 x 4 full-utilization [i=128, h=768, j=256] PE matmuls, the A/B linear
parts ride in as per-label rank-1 matmuls, and elementwise work happens
only on the small projection grids (u_k/v_k as parity polynomials in
t=s^2 on DVE, squares + all PSUM evacuations on ACT).

Sharding: 8 cores = 4 batches x 2 halves of the i axis; each core produces
a (128, 256, 13) output shard, stored n-major as 13 [128, 256] stores that
overlap the tail of the matmul stream.
"""

import sys

if "/opt/trn_rl_repo" not in sys.path:
    sys.path.insert(0, "/opt/trn_rl_repo")

import numpy as np

B = 4
L = 256
H = 768
NH = 6            # 128-partition chunks of H
NL = 13           # num labels
IH = 128          # i rows per core
R = 4             # residual rank

# Folds run on DVE for these n, gpsimd for the rest.
POOL_NS = (4, 9)

# Parity-structured cubic (in t=s^2) coefficients for u_k / v_k,
# from the offline SVD+ALS fit (fit_design.py, R=5 DEG=3, sigma=0.46).
UPAR = ["even", "odd", "even", "odd"]
VPAR = ["even", "odd", "even", "odd"]
UCOEF = [
    [-0.2603596652636215, -0.640638145631476, 0.06734414362633942, -0.00467700755409076],
    [-0.8786889970070607, 0.23169098694336426, -0.03833744685423369, 0.0028900170856760747],
    [0.2594939944026518, -0.8744674712375285, 0.15363158779322958, -0.014178377837107818],
    [0.15347940695644877, -0.3464424077242445, 0.09542566268640645, -0.009108033065574042],
]
VCOEF = [
    [-0.26035966527059107, -0.6406381456049605, 0.06734414362177382, -0.004677007553672564],
    [-0.8786889970388821, 0.23169098701368374, -0.03833744687358098, 0.0028900170875221076],
    [-0.2594939945034542, 0.8744674716222097, -0.1536315878610007, 0.014178377843414016],
    [0.15347941444821156, -0.3464424243413199, 0.09542566732018067, -0.009108033511683584],
]

_CACHE = {}


def _factor_cubic(coef):
    """c0+c1 t+c2 t^2+c3 t^3 = scale*(t - r)*(t^2 + p t + q), r real."""
    c0, c1, c2, c3 = coef
    roots = np.roots([c3, c2, c1, c0])
    rr = [x.real for x in roots if abs(x.imag) < 1e-9]
    r = min(rr, key=abs)
    b = c2 + c3 * r
    c = c1 + b * r
    return float(r), float(b / c3), float(c / c3), float(c3)


def _build(repeat=1, pool_ns=None, stt=False):
    if pool_ns is None:
        pool_ns = POOL_NS
    import concourse.mybir as mybir
    from concourse import bacc
    from concourse.tile import TileContext

    f32 = mybir.dt.float32
    bf16 = mybir.dt.bfloat16
    f32r = mybir.dt.float32r
    SQUARE = mybir.ActivationFunctionType.Square
    COPY = mybir.ActivationFunctionType.Copy
    IDENT = mybir.ActivationFunctionType.Identity
    MULT = mybir.AluOpType.mult
    ADD = mybir.AluOpType.add

    nc = bacc.Bacc("TRN2", target_bir_lowering=False)

    xt_d = nc.dram_tensor("xt", [128, NH * L], bf16, kind="ExternalInput")
    xts_d = nc.dram_tensor("xts", [128, NH * IH], bf16, kind="ExternalInput")
    w1s_d = nc.dram_tensor("w1s", [NH, 128, NH * 128], bf16, kind="ExternalInput")
    w1e_d = nc.dram_tensor("w1e", [NH, 128, NH * 128], bf16, kind="ExternalInput")
    b1t_d = nc.dram_tensor("b1t", [1, H], f32r, kind="ExternalInput")
    w2h_d = nc.dram_tensor("w2h", [128, NH * NL], bf16, kind="ExternalInput")
    w2pat_d = nc.dram_tensor("w2pat", [NL, 128, NH * 128], bf16, kind="ExternalInput")
    b2t_d = nc.dram_tensor("b2t", [NL, 1], f32, kind="ExternalInput")
    out_d = nc.dram_tensor("out", [IH, NL * L], f32, kind="ExternalOutput")

    with TileContext(nc) as tc:
        def body():
            with (
                tc.tile_pool(name="consts", bufs=1) as cp,
                tc.tile_pool(name="w1p", bufs=1) as wp,
                tc.tile_pool(name="evp", bufs=2) as ep,
                tc.tile_pool(name="ukp", bufs=3) as up,
                tc.tile_pool(name="fp", bufs=5) as fp,
                tc.tile_pool(name="obp", bufs=4) as op,
            ):
                # ACT table preload: a dummy Square fires the table-set
                # load at t~0 so the first real evac doesn't pay ~2.7us.
                warm = cp.tile([1, 16], f32, tag="warm", name="warm")
                nc.vector.memset(warm, 1.0)
                nc.scalar.activation(out=warm, in_=warm, func=SQUARE)

                # ---- input DMAs: few big transfers, S/E interleaved ----
                XTSf = cp.tile([128, NH * IH], bf16, tag="xtsf", name="XTSf")
                nc.sync.dma_start(out=XTSf, in_=xts_d[:, :])
                XTf = cp.tile([128, NH * L], bf16, tag="xtf", name="XTf")
                nc.sync.dma_start(out=XTf, in_=xt_d[:, :])
                B1R = cp.tile([1, H], f32r, tag="b1r", name="B1R")
                nc.sync.dma_start(out=B1R, in_=b1t_d[:, :])
                ONES = cp.tile([1, L], f32, tag="ones", name="ONES")
                nc.vector.memset(ONES, 1.0)
                XT = [XTf[:, h * L:(h + 1) * L] for h in range(NH)]
                XTS = [XTSf[:, h * IH:(h + 1) * IH] for h in range(NH)]

                # W1 in 4 half-slabs (3 kc-chunks each), S/E interleaved
                W1S_half = [None, None]
                W1E_half = [None, None]
                for hf in range(2):
                    tS = wp.tile([128, 3 * NH * 128], bf16, tag=f"w1s{hf}",
                                 name=f"W1SH{hf}")
                    nc.sync.dma_start(
                        out=tS.rearrange("p (k c) -> p k c", k=3),
                        in_=w1s_d[3 * hf:3 * hf + 3].rearrange("k p c -> p k c"),
                    )
                    W1S_half[hf] = tS
                for hf in range(2):
                    tE = wp.tile([128, 3 * NH * 128], bf16, tag=f"w1e{hf}",
                                 name=f"W1EH{hf}")
                    nc.sync.dma_start(
                        out=tE.rearrange("p (k c) -> p k c", k=3),
                        in_=w1e_d[3 * hf:3 * hf + 3].rearrange("k p c -> p k c"),
                    )
                    W1E_half[hf] = tE

                def w1_slabs(k):
                    hf, r_ = k // 3, k % 3
                    w = NH * 128
                    return (
                        W1E_half[hf][:, r_ * w:(r_ + 1) * w],
                        W1S_half[hf][:, r_ * w:(r_ + 1) * w],
                    )

                W2H = cp.tile([128, NH * NL], bf16, tag="w2h", name="W2H")
                nc.sync.dma_start(out=W2H, in_=w2h_d[:, :])
                B2T = cp.tile([NL, 1], f32, tag="b2t", name="B2T")
                nc.sync.dma_start(out=B2T, in_=b2t_d[:, :])
                W2Hc = [W2H[:, h * NL:(h + 1) * NL] for h in range(NH)]

                W2P = cp.tile([128, NL * NH * 128], bf16, tag="w2p", name="W2P")
                W2Pn = [W2P[:, n * NH * 128:(n + 1) * NH * 128] for n in range(NL)]

                # ---- projections: S=[h,i] (b1 folded), E=[h,j]; bf16 ----
                # (transient PSUM pool, closed before the 7 residual banks)
                pp0_cm = tc.tile_pool(name="pp0", bufs=2, space="PSUM")
                pp0 = pp0_cm.__enter__()
                Sbf = cp.tile([128, NH * IH], bf16, tag="sbf", name="Sbf")
                Ebf = cp.tile([128, NH * L], bf16, tag="ebf", name="Ebf")
                Sc = [Sbf[:, h * IH:(h + 1) * IH] for h in range(NH)]
                Ec = [Ebf[:, h * L:(h + 1) * L] for h in range(NH)]
                pxs_all = pp0.tile([128, NH * IH], f32, tag="pxs",
                                   bufs=1, name="pxs_all")
                # b1 rank-1s first: they only need tiny DMAs, run at ~2us,
                # and clear the PSUM banks (start=True on first bank touch).
                for k in range(NH):
                    nc.tensor.matmul(
                        pxs_all[:, k * IH:(k + 1) * IH],
                        lhsT=B1R[0:1, k * 128:(k + 1) * 128],
                        rhs=ONES[0:1, 0:IH].bitcast(f32r),
                        # [128, 768] f32 = 1.5 banks: chunks 0-3 share bank 0
                        start=(k % 4 == 0),
                        stop=False,
                        skip_group_check=True,
                    )
                for k in range(NH):
                    _, W1Sk = w1_slabs(k)
                    reg = pxs_all[:, k * IH:(k + 1) * IH]
                    for h in range(NH):
                        nc.tensor.matmul(
                            reg,
                            lhsT=W1Sk[:, h * 128:(h + 1) * 128],
                            rhs=XTS[h],
                            start=False,
                            stop=(k == NH - 1 and h == NH - 1),
                            skip_group_check=True,
                        )
                nc.scalar.activation(out=Sbf, in_=pxs_all, func=COPY)
                ts = ep.tile([128, NH * IH], bf16, tag="ts", bufs=1, name="ts")
                nc.scalar.activation(out=ts, in_=Sbf, func=SQUARE)
                ts2 = ep.tile([128, NH * IH], bf16, tag="ts2", bufs=1, name="ts2")
                nc.scalar.activation(out=ts2, in_=ts, func=SQUARE)
                pxe_all = pp0.tile([128, NH * L], f32, tag="pxe",
                                   bufs=1, name="pxe_all")
                for k in range(NH):
                    W1Ek, _ = w1_slabs(k)
                    reg = pxe_all[:, k * L:(k + 1) * L]
                    for h in range(NH):
                        nc.tensor.matmul(
                            reg,
                            lhsT=W1Ek[:, h * 128:(h + 1) * 128],
                            rhs=XT[h],
                            # [128, 1536] f32 = 3 banks: 2 chunks per bank
                            start=(h == 0 and k % 2 == 0),
                            stop=(k == NH - 1 and h == NH - 1),
                            skip_group_check=True,
                        )
                eh = NH * L // 2
                nc.scalar.activation(
                    out=Ebf[:, :eh], in_=pxe_all[:, :eh], func=COPY
                )
                te = ep.tile([128, NH * L], bf16, tag="te", bufs=1, name="te")
                nc.scalar.activation(out=te[:, :eh], in_=Ebf[:, :eh], func=SQUARE)
                te2 = ep.tile([128, NH * L], bf16, tag="te2", bufs=1, name="te2")
                nc.scalar.activation(out=te2[:, :eh], in_=te[:, :eh], func=SQUARE)
                nc.scalar.activation(
                    out=Ebf[:, eh:], in_=pxe_all[:, eh:], func=COPY
                )
                nc.scalar.activation(out=te[:, eh:], in_=Ebf[:, eh:], func=SQUARE)
                nc.scalar.activation(out=te2[:, eh:], in_=te[:, eh:], func=SQUARE)

                # W2 fold patterns (replicated over i): per-n DMAs issued
                # after the projection-critical loads so fold(k,n) only
                # waits for its own slice.
                for n in range(NL):
                    nc.sync.dma_start(out=W2Pn[n], in_=w2pat_d[n])

                # ---- linear parts, as [13, *] tiles for rank-1 re-adds ----
                pA = pp0.tile([NL, IH], f32, tag="pA", bufs=1, name="pA")
                for h in range(NH):
                    nc.tensor.matmul(
                        pA, lhsT=W2Hc[h], rhs=Sc[h],
                        start=(h == 0), stop=(h == NH - 1),
                    )
                Atmp = cp.tile([NL, IH], f32, tag="atmp", name="Atmp")
                nc.scalar.activation(out=Atmp, in_=pA, func=COPY)
                AO = cp.tile([2, NL * IH], f32, tag="a1", name="AO")
                nc.vector.memset(AO, 1.0)
                nc.sync.dma_start(
                    out=AO[0:1, :].rearrange("p (n i) -> p n i", n=NL), in_=Atmp
                )

                pB = pp0.tile([NL, L], f32, tag="pB", bufs=1, name="pB")
                for h in range(NH):
                    nc.tensor.matmul(
                        pB, lhsT=W2Hc[h], rhs=Ec[h],
                        start=(h == 0), stop=(h == NH - 1),
                    )
                Btmp = cp.tile([NL, L], f32, tag="btmp", name="Btmp")
                nc.scalar.activation(
                    out=Btmp, in_=pB, func=IDENT, bias=B2T[:, 0:1]
                )
                OB = cp.tile([2, NL * L], f32, tag="bt1", name="OB")
                nc.vector.memset(OB, 1.0)
                nc.sync.dma_start(
                    out=OB[1:2, :].rearrange("p (n j) -> p n j", n=NL), in_=Btmp
                )
                pp0_cm.__exit__(None, None, None)

                # ---- residual psums: 13 n-tiles packed 2 per PSUM bank ----
                ppn_cm = tc.tile_pool(name="ppn", bufs=1, space="PSUM")
                ppn = ppn_cm.__enter__()
                pbank = [
                    ppn.tile([128, 2 * L], f32, tag=f"pb{b_}", bufs=1,
                             name=f"pbank{b_}")
                    for b_ in range(7)
                ]
                psum_n = [pbank[n // 2][:, (n % 2) * L:(n % 2 + 1) * L]
                          for n in range(NL)]

                def poly_ops(dst, x, t, t2, coef, parity, pool, tag, w):
                    """Return a list of zero-arg closures, one DVE op each."""
                    ops = []
                    if stt:
                        # factored: p = ((t^2+pt) + q) * (c3 t - c3 r); odd *x
                        r_, p_, q_, a_ = _factor_cubic(coef)
                        w_ = pool.tile([128, w], bf16, tag=f"{tag}a", name=f"{tag}a")
                        ops.append(lambda c=None: nc.vector.scalar_tensor_tensor(
                                out=w_, in0=t, scalar=p_, in1=t2, op0=MULT, op1=ADD
                            ))
                        v_ = pool.tile([128, w], bf16, tag=f"{tag}b", name=f"{tag}b")
                        ops.append(lambda c=None: nc.vector.tensor_scalar(
                                out=v_, in0=t, scalar1=a_, scalar2=-a_ * r_,
                                op0=MULT, op1=ADD,
                            ))
                        if parity == "odd":
                            ops.append(lambda c=None: nc.vector.scalar_tensor_tensor(
                                    out=w_, in0=w_, scalar=q_, in1=v_,
                                    op0=ADD, op1=MULT,
                                ))
                            ops.append(lambda c=None: nc.vector.tensor_mul(out=dst, in0=w_, in1=x))
                            return ops
                        else:
                            ops.append(lambda c=None: nc.vector.scalar_tensor_tensor(
                                    out=dst, in0=w_, scalar=q_, in1=v_,
                                    op0=ADD, op1=MULT,
                                ))
                            return ops
                        return ops
                    # Estrin: p(t) = (c0 + c1 t) + t2*(c2 + c3 t); odd: *x
                    c0, c1, c2, c3 = coef
                    a1 = pool.tile([128, w], bf16, tag=f"{tag}a", name=f"{tag}a")
                    ops.append(lambda c=None: nc.vector.tensor_scalar(
                            out=a1, in0=t, scalar1=c1, scalar2=c0, op0=MULT, op1=ADD
                        ))
                    b1_ = pool.tile([128, w], bf16, tag=f"{tag}b", name=f"{tag}b")
                    ops.append(lambda c=None: nc.vector.tensor_scalar(
                            out=b1_, in0=t, scalar1=c3, scalar2=c2, op0=MULT, op1=ADD
                        ))
                    ops.append(lambda c=None: nc.vector.tensor_mul(out=b1_, in0=b1_, in1=t2))
                    if parity == "odd":
                        ops.append(lambda c=None: nc.vector.tensor_add(out=a1, in0=a1, in1=b1_))
                        ops.append(lambda c=None: nc.vector.tensor_mul(out=dst, in0=a1, in1=x))
                    else:
                        ops.append(lambda c=None: nc.vector.tensor_add(out=dst, in0=a1, in1=b1_))
                    return ops

                # ---- steady state over k; per-n close + store on k=R-1 ----
                def close_n(n):
                    # A[i,n] (+) B[j,n] in one K=2 matmul: lhsT rows are
                    # (A-col, ones), rhs rows are (ones, B-row).
                    nc.tensor.matmul(
                        psum_n[n],
                        lhsT=AO[:, n * IH:(n + 1) * IH].bitcast(f32r),
                        rhs=OB[:, n * L:(n + 1) * L].bitcast(f32r),
                        start=False, stop=True, skip_group_check=True,
                    )
                    obn = op.tile([128, L], f32, tag="ob", name=f"ob{n}")
                    nc.scalar.activation(out=obn, in_=psum_n[n], func=COPY)
                    nc.sync.dma_start(
                        out=out_d[:, n * L:(n + 1) * L], in_=obn
                    )

                def make_u(k):
                    uk = up.tile([128, NH * IH], bf16, tag="uk", name=f"uk{k}")
                    ops = poly_ops(uk, Sbf, ts, ts2, UCOEF[k], UPAR[k], up,
                                   "ue", NH * IH)
                    return uk, ops

                def make_v(k):
                    vk = up.tile([128, NH * L], bf16, tag=f"vk{k}", bufs=1,
                                 name=f"vk{k}")
                    ops = poly_ops(vk, Ebf, te, te2, VCOEF[k], VPAR[k], up,
                                   "ve", NH * L)
                    return vk, ops

                def fold(k, n, uk):
                    ukn = fp.tile([128, NH * IH], bf16, tag="ukn",
                                  name=f"ukn{k}_{n}")
                    eng = nc.gpsimd if n in pool_ns else nc.vector
                    eng.tensor_mul(out=ukn, in0=uk, in1=W2Pn[n])
                    return ukn

                uk0, uops = make_u(0)
                for f_ in uops:
                    f_()
                ukn00 = fold(0, 0, uk0)
                vk0 = up.tile([128, NH * L], bf16, tag="vk0", bufs=1,
                              name="vk0")
                for hf_ in range(2):
                    sl = slice(hf_ * eh, (hf_ + 1) * eh)
                    for f_ in poly_ops(vk0[:, sl], Ebf[:, sl], te[:, sl],
                                       te2[:, sl], VCOEF[0], VPAR[0], up,
                                       f"v0h{hf_}", eh):
                        f_()
                uv = (uk0, vk0)
                pending = []
                for k in range(R):
                    uk, vk = uv
                    for n in range(NL):
                        if n == 1 and k + 1 < R:
                            # software-pipeline: next k's eval ops drip in
                            # one per fold so PE never starves at the
                            # k boundary.
                            uk1, uo = make_u(k + 1)
                            vk1, vo = make_v(k + 1)
                            pending = uo + vo
                            uv = (uk1, vk1)
                        ukn = ukn00 if (k == 0 and n == 0) else fold(k, n, uk)
                        if pending:
                            pending.pop(0)()
                        for c in range(NH):
                            nc.tensor.matmul(
                                psum_n[n],
                                lhsT=ukn[:, c * IH:(c + 1) * IH],
                                rhs=vk[:, c * L:(c + 1) * L],
                                start=(k == 0 and c == 0 and n % 2 == 0),
                                stop=False,
                                skip_group_check=True,
                            )
                        if k == R - 1:
                            close_n(n)
                    for f_ in pending:
                        f_()
                    pending = []

                ppn_cm.__exit__(None, None, None)

        if repeat == 1:
            body()
        else:
            with tc.For_i(0, repeat, 1):
                body()

    nc.compile()
    return nc


def _get_program(repeat=1, **kw):
    key = (repeat, tuple(sorted(kw.items())))
    if key not in _CACHE:
        _CACHE[key] = _build(repeat, **kw)
    return _CACHE[key]


def make_in_maps(hidden_states, W1, b1, W2, b2):
    hidden_states = np.asarray(hidden_states, dtype=np.float32)
    W1 = np.asarray(W1, dtype=np.float32)
    b1 = np.asarray(b1, dtype=np.float32)
    W2 = np.asarray(W2, dtype=np.float32)
    b2 = np.asarray(b2, dtype=np.float32)

    import ml_dtypes

    bf = ml_dtypes.bfloat16

    def w1_prep(w):
        # [(c p), (k kk)] -> [k, p, (c kk)]: per-kc slab, direct tile layout.
        return np.ascontiguousarray(
            w.reshape(NH, 128, NH, 128).transpose(2, 1, 0, 3).reshape(NH, 128, NH * 128)
        ).astype(bf)

    w1s = w1_prep(W1[:H])
    w1e = w1_prep(W1[H:])
    b1t = np.ascontiguousarray(b1.reshape(1, H))
    # 0.5*W2 chunks [h-part, (c,n)] for the linear matmuls
    w2h = np.ascontiguousarray(
        (0.5 * W2).reshape(NH, 128, NL).transpose(1, 0, 2).reshape(128, NH * NL)
    ).astype(bf)
    # fold patterns: w2pat[n, p, c*128+i] = W2[c*128+p, n]
    w2pat = np.ascontiguousarray(
        np.broadcast_to(
            W2.reshape(NH, 128, NL).transpose(2, 1, 0)[:, :, :, None],
            (NL, 128, NH, 128),
        ).reshape(NL, 128, NH * 128)
    ).astype(bf)
    b2t = np.ascontiguousarray(b2.reshape(NL, 1))

    in_maps = []
    for core in range(8):
        b, ih = core // 2, core % 2
        xt = np.ascontiguousarray(
            hidden_states[b].reshape(L, NH, 128).transpose(2, 1, 0).reshape(128, NH * L)
        ).astype(bf)
        xts = np.ascontiguousarray(
            hidden_states[b][ih * IH:(ih + 1) * IH]
            .reshape(IH, NH, 128).transpose(2, 1, 0).reshape(128, NH * IH)
        ).astype(bf)
        in_maps.append(
            {
                "xt": xt,
                "xts": xts,
                "w1s": w1s,
                "w1e": w1e,
                "b1t": b1t,
                "w2h": w2h,
                "w2pat": w2pat,
                "b2t": b2t,
            }
        )
    return in_maps


def kernel(hidden_states, W1, b1, W2, b2):
    from concourse.bass_utils import run_bass_kernel_spmd

    nc = _get_program()
    in_maps = make_in_maps(hidden_states, W1, b1, W2, b2)
    res = run_bass_kernel_spmd(nc, in_maps, core_ids=list(range(8)))

    out = np.empty((B, L, L, NL), dtype=np.float32)
    for core in range(8):
        b, ih = core // 2, core % 2
        out[b, ih * IH:(ih + 1) * IH] = (
            res.results[core]["out"].reshape(IH, NL, L).transpose(0, 2, 1)
        )
    return out

